# revision 62
# baseline (speedup 1.0000x reference)
"""Trainium2 Bass kernel for nn_BDH_69638599737422 (dense_transformer).

Sharding (8 NeuronCores): core c = 2*h + j owns head h (of 4) and N-half j
(4096 of 8192 latent dims). encoder/encoder_v column-parallel, decoder
row-parallel. Per layer: one 2-rank AllGather (partial yKV within a head
pair, since scores contract over the full head N) and one 8-rank AllReduce
(y = xy @ decoder partial sums into D).

All on-device tensors are fp16 (PE matmuls run fp16 at full rate with fp32
PSUM accumulation; ~1.4e-3 rel err vs the fp32 reference).

The RoPE frequency table repeats in pairs (quantize(t,2)), so a host-side
even/odd de-interleave permutation of each core's N slice (baked into
encoder/encoder_v columns and decoder rows) turns rotate_half into two
contiguous halves: qe = xe*c - xo*s, qo = xo*c + xe*s.

The causal mask (strict lower) is applied on the transposed score matrix
P[s,t] = scores[t,s]: Q@Q^T is symmetric, so P comes out of the same
matmuls and the mask becomes strict-upper, which lets the per-s-chunk
matmuls skip the all-zero left region entirely (triangle skip).

Pipeline structure (everything is split into t-halves h0/h1 so each
8-core AllReduce window is filled with the next t-half's work):
  AR-h1 lands -> x-upd h1 -> phase1 h1 || rope h1 (streamed per 2-tile
  chunk) -> P right-quadrant (streamed) -> ykv tcn 2,3 -> AG h1 ->
  phase5 h1 -> AR-h1' ... while phase1 h0 / rope h0 / P left-quadrant /
  ykv tcn 0,1 / AG h0 / phase5 h0 ride inside the AR windows. The ykv
  matmuls skip s-chunks > t-chunk (strict-lower zero blocks), which also
  makes ykv tcn 0,1 depend only on the left P quadrant and the h0 half
  of x, so AG h0 flies ~15us earlier than a full-P schedule would allow.
"""

import os

import numpy as np

import concourse.bass as bass
import concourse.tile as tile
from concourse import bacc, mybir
from concourse.bass_utils import run_bass_kernel_spmd
from concourse.masks import make_identity

F16 = mybir.dt.float16
BF16 = mybir.dt.bfloat16
F32 = mybir.dt.float32
AF = mybir.ActivationFunctionType
ALU = mybir.AluOpType

B, T, D, NH, VOCAB = 1, 512, 256, 4, 256
N = 8192        # latent dim per head
NL = 4096       # per-core N slice
NPAIR = 2048    # rope pairs per core
NT = NL // 128  # 32 n-tiles per core
N_LAYER = 6
EPS = 1e-5
THETA = 2.0 ** 16
TWO_PI = 2.0 * np.pi
CORES = list(range(8))
PAIR_GROUPS = [[0, 1], [2, 3], [4, 5], [6, 7]]

_STATE = {}


# ---------------------------------------------------------------- host prep

def _ln_np(x):
    m = x.mean(-1, keepdims=True)
    v = ((x - m) ** 2).mean(-1, keepdims=True)
    return (x - m) / np.sqrt(v + EPS)


def _rope_pair_tables():
    """cos/sin at even lanes only (freqs repeat in pairs): [T, N//2] f32."""
    t = np.arange(N, dtype=np.float32)
    q = (np.floor(t / 2.0) * 2.0).astype(np.float32)
    freqs = (1.0 / (THETA ** (q / np.float32(N))) / np.float32(TWO_PI)).astype(
        np.float32
    )
    pos = np.arange(T, dtype=np.float32)
    ang = ((pos[:, None] * freqs[None, :]) % 1.0) * np.float32(TWO_PI)
    cos = np.cos(ang).astype(np.float32)
    sin = np.sin(ang).astype(np.float32)
    return cos[:, ::2], sin[:, ::2]


def _tileize_rows(a, rows_per_tile=128):
    """[n_tiles*128, w] -> [128, n_tiles*w] with free dim = (tile, w)."""
    r, w = a.shape
    nt = r // rows_per_tile
    return np.ascontiguousarray(
        a.reshape(nt, rows_per_tile, w).transpose(1, 0, 2).reshape(rows_per_tile, nt * w)
    )


def _build_in_maps(idx, embed, encoder, encoder_v, decoder, lm_head):
    idx = np.asarray(idx)
    embed = np.asarray(embed, dtype=np.float32)
    encoder = np.asarray(encoder, dtype=np.float32)
    encoder_v = np.asarray(encoder_v, dtype=np.float32)
    decoder = np.asarray(decoder, dtype=np.float32)
    lm_head = np.asarray(lm_head, dtype=np.float32)

    x0 = _ln_np(embed[idx[0]]).astype(np.float16)          # [T, D]
    x_td0 = _tileize_rows(x0)                               # [128, 4*256]
    x_dt0 = _tileize_rows(np.ascontiguousarray(x0.T))       # [128, 2*512]

    cos_p, sin_p = _rope_pair_tables()                      # [T, 4096] f32
    # even lanes first, then odd lanes
    perm = np.concatenate([np.arange(0, NL, 2), np.arange(1, NL, 2)])

    maskd = np.triu(np.ones((128, 128), np.float16), k=1)   # keep s < t
    lmh = _tileize_rows(lm_head.astype(np.float16))         # [128, 2*256]

    in_maps = []
    for c in CORES:
        h, j = c // 2, c % 2
        nsl = slice(j * NL, (j + 1) * NL)
        enc_s = encoder[h][:, nsl][:, perm].astype(np.float16)      # [256, 4096]
        ev_s = encoder_v[h][:, nsl][:, perm].astype(np.float16)
        dec_s = decoder[h * N + j * NL : h * N + (j + 1) * NL][perm].astype(
            np.float16
        )                                                            # [4096, 256]
        kp = slice(j * NPAIR, (j + 1) * NPAIR)
        cos_s = np.ascontiguousarray(cos_p[:, kp].T).astype(np.float16)  # [2048, 512]
        sin_s = np.ascontiguousarray(sin_p[:, kp].T).astype(np.float16)
        in_maps.append(
            {
                "enc0": np.ascontiguousarray(enc_s[:128]),
                "enc1": np.ascontiguousarray(enc_s[128:]),
                "ev0": np.ascontiguousarray(ev_s[:128]),
                "ev1": np.ascontiguousarray(ev_s[128:]),
                "decb": _tileize_rows(dec_s),               # [128, 32*256]
                "cosb": _tileize_rows(cos_s),               # [128, 16*512]
                "sinb": _tileize_rows(sin_s),
                "maskd": maskd,
                "x_td0": x_td0,
                "x_dt0": x_dt0,
                "lmh": lmh,
            }
        )
    return in_maps


# ---------------------------------------------------------------- device code

def _ln_chunk(nc, st, out_f16, in_ap, tc, chunk, epst):
    """LN over one free-dim chunk: out = (in - mu) * rstd."""
    sl = slice(tc * chunk, (tc + 1) * chunk)
    stats = st.tile([128, 6], F32, tag="st6", name="stats")
    mv = st.tile([128, 2], F32, tag="st2", name="mv")
    nc.vector.bn_stats(out=stats, in_=in_ap[:, sl])
    nc.vector.bn_aggr(out=mv, in_=stats)
    nc.scalar.activation(
        out=mv[:, 1:2], in_=mv[:, 1:2], func=AF.Sqrt, bias=epst, scale=1.0
    )
    nc.vector.reciprocal(out=mv[:, 1:2], in_=mv[:, 1:2])
    nc.vector.tensor_scalar(
        out=out_f16[:, sl],
        in0=in_ap[:, sl],
        scalar1=mv[:, 0:1],
        scalar2=mv[:, 1:2],
        op0=ALU.subtract,
        op1=ALU.mult,
    )


def _build_bass():
    nc = bacc.Bacc(None, target_bir_lowering=False, num_devices=len(CORES))

    dp = nc.declare_dram_parameter
    enc0_e = dp("enc0", [128, NL], F16, isOutput=False)
    enc1_e = dp("enc1", [128, NL], F16, isOutput=False)
    ev0_e = dp("ev0", [128, NL], F16, isOutput=False)
    ev1_e = dp("ev1", [128, NL], F16, isOutput=False)
    dec_e = dp("decb", [128, NT * D], F16, isOutput=False)
    cos_e = dp("cosb", [128, 16 * T], F16, isOutput=False)
    sin_e = dp("sinb", [128, 16 * T], F16, isOutput=False)
    mask_e = dp("maskd", [128, 128], F16, isOutput=False)
    xtd_e = dp("x_td0", [128, 4 * D], F16, isOutput=False)
    xdt_e = dp("x_dt0", [128, 2 * T], F16, isOutput=False)
    lmh_e = dp("lmh", [128, 2 * VOCAB], F16, isOutput=False)
    out_e = dp("logits", [T, VOCAB], F32, isOutput=True)
    dbg = {}
    if os.environ.get("KERNEL_DEBUG"):
        dbg["xs"] = dp("dbg_xs", [128, NT * T], F16, isOutput=True)
        dbg["qr"] = dp("dbg_qr", [128, NT * T], F16, isOutput=True)
        dbg["Pb"] = dp("dbg_Pb", [128, 4 * T], F16, isOutput=True)
        dbg["yk0"] = dp("dbg_yk0", [128, 512], F16, isOutput=True)
        dbg["yk1"] = dp("dbg_yk1", [128, 512], F16, isOutput=True)
        dbg["ykdt0"] = dp("dbg_ykdt0", [128, 512], F16, isOutput=True)
        dbg["ykdt1"] = dp("dbg_ykdt1", [128, 512], F16, isOutput=True)
        dbg["xy"] = dp("dbg_xy", [128, NT * T], F16, isOutput=True)
        dbg["ysum0"] = dp("dbg_ysum0", [128, 512], F16, isOutput=True)
        dbg["ysum1"] = dp("dbg_ysum1", [128, 512], F16, isOutput=True)
        dbg["st2_0"] = dp("dbg_st2_0", [128, 512], F16, isOutput=True)
        dbg["st2_1"] = dp("dbg_st2_1", [128, 512], F16, isOutput=True)
        dbg["xtd1"] = dp("dbg_xtd1", [128, 4 * D], F16, isOutput=True)

    with tile.TileContext(nc) as tc_:
        pools = [
            tc_.tile_pool(name="wt", bufs=1),
            tc_.tile_pool(name="big", bufs=1),
            tc_.tile_pool(name="xp", bufs=2),
            tc_.tile_pool(name="tmp", bufs=2),
            tc_.tile_pool(name="ys", bufs=3),
            tc_.tile_pool(name="st", bufs=8),
            tc_.tile_pool(name="stg", bufs=1),
            tc_.tile_pool(name="ps", bufs=2, space="PSUM"),
            tc_.tile_pool(name="dram", bufs=2, space="DRAM"),
        ]
        wt, big, xp, tmp, ysp, st, stg, ps, dram = [p.__enter__() for p in pools]
        try:
            _emit(nc, wt, big, xp, tmp, ysp, st, stg, ps, dram,
                  enc0_e, enc1_e, ev0_e, ev1_e, dec_e, cos_e, sin_e, mask_e,
                  xtd_e, xdt_e, lmh_e, out_e, dbg)
        finally:
            for p in reversed(pools):
                p.__exit__(None, None, None)
    nc.compile()
    return nc


def _emit(nc, wt, big, xp, tmp, ysp, st, stg, ps, dram,
          enc0_e, enc1_e, ev0_e, ev1_e, dec_e, cos_e, sin_e, mask_e,
          xtd_e, xdt_e, lmh_e, out_e, dbg={}):
    dma = nc.sync.dma_start

    # persistent weights / tables
    enc0 = wt.tile([128, NL], F16, tag="enc0")
    enc1 = wt.tile([128, NL], F16, tag="enc1")
    ev0 = wt.tile([128, NL], F16, tag="ev0")
    ev1 = wt.tile([128, NL], F16, tag="ev1")
    dect = wt.tile([128, NT * D], F16, tag="dect")
    cost = wt.tile([128, 16 * T], F16, tag="cost")
    sint = wt.tile([128, 16 * T], F16, tag="sint")
    maskt = wt.tile([128, 128], F16, tag="maskt")
    lmht = wt.tile([128, 2 * VOCAB], F16, tag="lmht")
    ident = wt.tile([128, 128], F16, tag="ident")
    epst = wt.tile([128, 1], F32, tag="epst")

    xsb = big.tile([128, NT * T], F16, tag="xsb")    # xs then xy, (i, t)
    qrb = big.tile([128, NT * T], F16, tag="qrb")    # roped qs, (i, t)
    Pb = big.tile([128, 4 * T], F16, tag="Pb")       # masked scores^T, (sc, t)

    # tiny warmup collectives FIRST: their bounce DMAs precede the weight
    # DMAs in queue order, so the ~65-90us first-collective ncfw setup runs
    # concurrently with the input streaming + phase 1 instead of after it.
    wup = stg.tile([128, 16], F16, tag="wup")
    nc.vector.memset(wup, 0.0)
    wag_i = dram.tile([128, 8], F32, tag="wag_i")
    wag_o = dram.tile([2, 128, 8], F32, tag="wag_o")
    war_i = dram.tile([128, 16], F16, tag="war_i")
    war_o = dram.tile([128, 16], F16, tag="war_o")
    dma(out=wag_i[:].bitcast(F16), in_=wup)
    dma(out=war_i, in_=wup)
    nc.gpsimd.collective_compute(
        "AllGather", ALU.bypass, replica_groups=PAIR_GROUPS,
        ins=[wag_i.opt()], outs=[wag_o.opt()],
    )
    nc.gpsimd.collective_compute(
        "AllReduce", ALU.add, replica_groups=[CORES],
        ins=[war_i.opt()], outs=[war_o.opt()],
    )

    # input DMAs ordered by first use: phase 1 (x, enc), rope (cos/sin,
    # split so early chunks unblock before the full table lands), P (mask),
    # then phase 5 / decoder / lm_head.
    x_first = xp.tile([128, 4 * D], F16, tag="x_td")
    xd_first = xp.tile([128, 2 * T], F16, tag="x_dt")
    dma(out=xd_first, in_=xdt_e[:])
    dma(out=x_first, in_=xtd_e[:])
    dma(out=enc0, in_=enc0_e[:])
    dma(out=enc1, in_=enc1_e[:])
    dma(out=cost[:, : 8 * T], in_=cos_e[:, : 8 * T])
    dma(out=sint[:, : 8 * T], in_=sin_e[:, : 8 * T])
    dma(out=cost[:, 8 * T :], in_=cos_e[:, 8 * T :])
    dma(out=sint[:, 8 * T :], in_=sin_e[:, 8 * T :])
    dma(out=maskt, in_=mask_e[:])
    dma(out=ev0, in_=ev0_e[:])
    dma(out=ev1, in_=ev1_e[:])
    dma(out=dect, in_=dec_e[:])
    dma(out=lmht, in_=lmh_e[:])
    nc.vector.memset(epst, EPS)
    make_identity(nc, ident[:])

    # 3D views: [128, tile, t]
    xsb3 = xsb[:].rearrange("p (i t) -> p i t", t=T)
    qrb3 = qrb[:].rearrange("p (i t) -> p i t", t=T)
    cost3 = cost[:].rearrange("p (i t) -> p i t", t=T)
    sint3 = sint[:].rearrange("p (i t) -> p i t", t=T)
    # pair view for the phase-1 relu: [:, j, k, tcols] is tile j*16+k
    xsb4 = xsb[:].rearrange("p (j i t) -> p j i t", j=2, t=T)

    # rope chunk c covers tiles {2c, 2c+1} (even lanes) and {16+2c, 16+2c+1}
    # (odd); P consumes jt in the same order.
    jt_stream = []
    for c in range(8):
        jt_stream += [2 * c, 2 * c + 1, 16 + 2 * c, 16 + 2 * c + 1]

    def phase1_half(h, x_dt_ap):
        hsl = slice(h * 256, h * 256 + 256)
        for k in range(16):
            mm = ps.tile([128, 512], F32, tag="mm", bufs=2, name="mm1")
            for j, i in enumerate((k, 16 + k)):
                csl = slice(j * 256, (j + 1) * 256)
                nc.tensor.matmul(
                    out=mm[:, csl], lhsT=enc0[:, i * 128 : (i + 1) * 128],
                    rhs=x_dt_ap[:, 0 * T : 1 * T][:, hsl], start=True,
                    stop=False, skip_group_check=True,
                )
                nc.tensor.matmul(
                    out=mm[:, csl], lhsT=enc1[:, i * 128 : (i + 1) * 128],
                    rhs=x_dt_ap[:, 1 * T : 2 * T][:, hsl], start=False,
                    stop=True, skip_group_check=True,
                )
            nc.scalar.activation(
                out=xsb4[:, :, k, h * 256 : (h + 1) * 256],
                in_=mm[:].rearrange("p (j t) -> p j t", t=256),
                func=AF.Relu,
            )

    def rope_half(hp, mid_hook=None, hook_at=4):
        tsl = slice(hp * 256, (hp + 1) * 256)
        for c in range(8):
            if c == hook_at and mid_hook is not None:
                mid_hook()
            e2 = slice(2 * c, 2 * c + 2)
            o2 = slice(16 + 2 * c, 16 + 2 * c + 2)
            xe, xo = xsb3[:, e2, tsl], xsb3[:, o2, tsl]
            qe, qo = qrb3[:, e2, tsl], qrb3[:, o2, tsl]
            ct, st_ = cost3[:, e2, tsl], sint3[:, e2, tsl]
            tme = tmp.tile([128, 512], F16, tag="tmpe", name="tme")
            t3 = tme[:].rearrange("p (i t) -> p i t", t=256)
            nc.vector.tensor_mul(t3, xo, st_)
            nc.vector.tensor_mul(qe, xe, ct)
            nc.vector.tensor_sub(qe, qe, t3)
            tmo = tmp.tile([128, 512], F16, tag="tmpo", name="tmo")
            t4 = tmo[:].rearrange("p (i t) -> p i t", t=256)
            nc.vector.tensor_mul(t4, xe, st_)
            nc.vector.tensor_mul(qo, xo, ct)
            nc.vector.tensor_add(qo, qo, t4)

    def alloc_P():
        return [
            ps.tile([128, T], F32, tag="Pps", bufs=4, name=f"P_ps{m}")
            for m in range(4)
        ]

    def P_left(P_ps):
        for n, jt in enumerate(jt_stream):
            b = jt * T
            nc.tensor.matmul(
                out=P_ps[0][:, 0:256], lhsT=qrb[:, b : b + 128],
                rhs=qrb[:, b : b + 256], start=(n == 0), stop=(n == 31),
                skip_group_check=True,
            )
            nc.tensor.matmul(
                out=P_ps[1][:, 128:256], lhsT=qrb[:, b + 128 : b + 256],
                rhs=qrb[:, b + 128 : b + 256], start=(n == 0), stop=(n == 31),
                skip_group_check=True,
            )

    def P_right(P_ps):
        for n, jt in enumerate(jt_stream):
            b = jt * T
            for m in range(4):
                lo = max(256, m * 128)
                nc.tensor.matmul(
                    out=P_ps[m][:, lo:512],
                    lhsT=qrb[:, b + m * 128 : b + (m + 1) * 128],
                    rhs=qrb[:, b + lo : b + 512], start=(n == 0),
                    stop=(n == 31), skip_group_check=True,
                )

    def mask_copy_left(P_ps):
        nc.vector.tensor_mul(Pb[:, 0:128], P_ps[0][:, 0:128], maskt)
        nc.vector.tensor_mul(Pb[:, T + 128 : T + 256], P_ps[1][:, 128:256], maskt)
        nc.scalar.copy(out=Pb[:, 128:256], in_=P_ps[0][:, 128:256])

    def mask_copy_right(P_ps):
        nc.vector.tensor_mul(
            Pb[:, 2 * T + 256 : 2 * T + 384], P_ps[2][:, 256:384], maskt
        )
        nc.vector.tensor_mul(
            Pb[:, 3 * T + 384 : 3 * T + 512], P_ps[3][:, 384:512], maskt
        )
        nc.scalar.copy(out=Pb[:, 256:512], in_=P_ps[0][:, 256:512])
        nc.scalar.copy(out=Pb[:, T + 256 : T + 512], in_=P_ps[1][:, 256:512])
        nc.scalar.copy(out=Pb[:, 2 * T + 384 : 2 * T + 512], in_=P_ps[2][:, 384:512])

    def ykv_ag(h, x_td_ap):
        """ykv partial for t-chunks {2h, 2h+1} + pair AllGather trigger.
        Skips s-chunks > t-chunk (strict-lower zeros); for h=0 this only
        reads the left P quadrant and the h0 half of x_td."""
        ykv_ps = ps.tile([128, 512], F32, tag="acc", bufs=2, name=f"ykv_ps{h}")
        for k in range(2):
            tcn = 2 * h + k
            scs = list(range(tcn + 1))
            for idx, sc in enumerate(scs):
                nc.tensor.matmul(
                    out=ykv_ps[:, k * 256 : (k + 1) * 256],
                    lhsT=Pb[:, sc * T + tcn * 128 : sc * T + (tcn + 1) * 128],
                    rhs=x_td_ap[:, sc * D : (sc + 1) * D],
                    start=(idx == 0), stop=(idx == len(scs) - 1),
                    skip_group_check=True,
                )
        # 1/64 pre-scale keeps the pair-sum inside fp16 range; the LN that
        # follows is scale-invariant so this is exact.
        stage1 = stg.tile([128, 512], F16, tag=f"stg1_{h}", name=f"stg1_{h}")
        nc.vector.tensor_scalar_mul(out=stage1, in0=ykv_ps, scalar1=0.015625)
        b1i = dram.tile([128, D], F32, tag=f"b1i{h}", name=f"b1i{h}")
        b1o = dram.tile([2, 128, D], F32, tag=f"b1o{h}", name=f"b1o{h}")
        dma(out=b1i[:].bitcast(F16), in_=stage1)
        nc.gpsimd.collective_compute(
            "AllGather", ALU.bypass, replica_groups=PAIR_GROUPS,
            ins=[b1i.opt()], outs=[b1o.opt()],
        )
        return b1o, ykv_ps

    def pacer(gate, out_ps):
        """Burst of dummy matmuls gated on `gate` writing a dead PSUM
        region: keeps the PE HAM activity window busy through engine-idle
        latency gaps. A single tiny matmul is not enough activity — the
        throttle still re-engages and the next ~40 real matmuls run at
        half clock; a ~0.7us burst per chain step prevents that."""
        if out_ps is None:
            return
        nc.tensor.matmul(
            out=out_ps, lhsT=ident, rhs=gate, start=True, stop=True,
            skip_group_check=True,
        )

    def ykv_sum_ln(h, b1o, pace=None):
        """pair-sum + LN (DVE/ACT only; no PE ops so it can ride mid-rope
        without pushing transposes ahead of P in the PE queue)."""
        ykvsum = stg.tile([128, 512], F16, tag=f"ykvsum{h}", name=f"ykvsum{h}")
        agt = stg.tile([128, 512], F16, tag=f"agt{h}", name=f"agt{h}")
        dma(out=ykvsum, in_=b1o[0].bitcast(F16))
        dma(out=agt, in_=b1o[1].bitcast(F16))
        nc.vector.tensor_add(ykvsum, ykvsum, agt)
        pacer(ykvsum[:, 0:256], pace)
        ykv_td = xp.tile([128, 512], F16, tag=f"ykv_td{h}", name=f"ykv_td{h}")
        for k in range(2):
            _ln_chunk(nc, st, ykv_td, ykvsum, k, D, epst)
            pacer(ykv_td[:, k * 256 : (k + 1) * 256], pace)
        return ykv_td

    def ykv_tr(h, ykv_td):
        """transpose into ykv_dt_h [128, (dc, 256)] (PE + ACT)."""
        ykv_dt = xp.tile([128, 512], F16, tag=f"ykv_dt{h}", name=f"ykv_dt{h}")
        for k in range(2):
            for dc in range(2):
                tr = ps.tile([128, 128], F16, tag="mm", bufs=2, name="tr")
                nc.tensor.transpose(
                    tr, ykv_td[:, k * D + dc * 128 :][:, :128], ident
                )
                nc.scalar.copy(
                    out=ykv_dt[:, dc * 256 + k * 128 :][:, :128], in_=tr
                )
        return ykv_dt

    def phase5(h, ykv_dt, mid_hook=None):
        """ys = relu(yKV @ encv); xy = xs*ys in place; y += xy @ dec;
        AllReduce trigger for this half's y partials. mid_hook emits
        DVE-only work into the xy stream once its gating collective has
        landed (rides the queue without stalling it)."""
        # two separate PSUM slots: interleaving two open accumulation
        # groups inside one bank corrupts the first group's columns.
        y_ps = [
            ps.tile([128, 256], F32, tag="acc", bufs=2, name=f"y_ps{h}_{k}")
            for k in range(2)
        ]
        for g in range(NT // 4):
            if g == 5 and mid_hook is not None:
                mid_hook()
            ys_grp = ysp.tile([128, 4 * 256], F16, tag="ys", name="ys_grp")
            for half in range(2):
                mm = ps.tile([128, 512], F32, tag="mm", bufs=2, name="mm5")
                for j in range(2):
                    i = 4 * g + 2 * half + j
                    csl = slice(j * 256, (j + 1) * 256)
                    nc.tensor.matmul(
                        out=mm[:, csl], lhsT=ev0[:, i * 128 : (i + 1) * 128],
                        rhs=ykv_dt[:, 0:256],
                        start=True, stop=False, skip_group_check=True,
                    )
                    nc.tensor.matmul(
                        out=mm[:, csl], lhsT=ev1[:, i * 128 : (i + 1) * 128],
                        rhs=ykv_dt[:, 256:512],
                        start=False, stop=True, skip_group_check=True,
                    )
                nc.scalar.activation(
                    out=ys_grp[:, half * 512 : (half + 1) * 512], in_=mm,
                    func=AF.Relu,
                )
            xs_grp = (
                xsb[:, 4 * g * T : 4 * (g + 1) * T]
                .rearrange("p (i t) -> p i t", t=T)[:, :, h * 256 : (h + 1) * 256]
            )
            nc.vector.tensor_mul(
                xs_grp, xs_grp,
                ys_grp.rearrange("p (i t) -> p i t", t=256),
            )
            for k4 in range(4):
                i = 4 * g + k4
                for k in range(2):
                    tcn = 2 * h + k
                    nc.tensor.matmul(
                        out=y_ps[k],
                        lhsT=xsb[:, i * T + tcn * 128 : i * T + (tcn + 1) * 128],
                        rhs=dect[:, i * D : (i + 1) * D],
                        start=(i == 0), stop=(i == NT - 1),
                        skip_group_check=True,
                    )
        stage2 = stg.tile([128, 512], F16, tag=f"stg2_{h}", name=f"stg2_{h}")
        for k in range(2):
            nc.scalar.copy(out=stage2[:, k * 256 : (k + 1) * 256], in_=y_ps[k])
        pacer(stage2[:, 0:256], y_ps[0])
        if dbg and dbg.get("_dump_st2"):
            dma(out=dbg[f"st2_{h}"][:], in_=stage2)
        b2i = dram.tile([128, 512], F16, tag=f"b2i{h}", name=f"b2i{h}")
        b2o = dram.tile([128, 512], F16, tag=f"b2o{h}", name=f"b2o{h}")
        dma(out=b2i, in_=stage2)
        nc.gpsimd.collective_compute(
            "AllReduce", ALU.add, replica_groups=[CORES],
            ins=[b2i.opt()], outs=[b2o.opt()],
        )
        return b2o, y_ps

    def x_upd_ln(h, b2o, x_td_old, x_td_new, dbg_key=None, pace=None):
        """y = LN(ysum); x_new = LN(x_old + y) (DVE/ACT only)."""
        hsl = slice(h * 2 * D, (h + 1) * 2 * D)
        ysum = stg.tile([128, 512], F16, tag=f"ysum{h}", name=f"ysum{h}")
        dma(out=ysum, in_=b2o)
        if dbg_key is not None:
            dma(out=dbg[dbg_key][:], in_=ysum)
        y_ln = stg.tile([128, 512], F16, tag=f"y_ln{h}", name=f"y_ln{h}")
        z = stg.tile([128, 512], F16, tag=f"z{h}", name=f"z{h}")
        for k in range(2):
            _ln_chunk(nc, st, y_ln, ysum, k, D, epst)
            pacer(y_ln[:, k * 256 : (k + 1) * 256], pace)
        nc.vector.tensor_add(z, y_ln, x_td_old[:, hsl])
        pacer(z[:, 0:256], pace)
        zv = z[:]
        xt4 = x_td_new[:, hsl]
        for k in range(2):
            _ln_chunk(nc, st, xt4, zv, k, D, epst)
            pacer(xt4[:, k * 256 : (k + 1) * 256], pace)

    def x_upd_tr(h, x_td_new, x_dt_new):
        """refresh x_dt for this half (PE transposes + ACT copies)."""
        for k in range(2):
            tcn = 2 * h + k
            for dc in range(2):
                tr = ps.tile([128, 128], F16, tag="mm", bufs=2, name="trx")
                nc.tensor.transpose(
                    tr, x_td_new[:, tcn * D + dc * 128 :][:, :128], ident
                )
                nc.scalar.copy(
                    out=x_dt_new[:, dc * T + tcn * 128 :][:, :128], in_=tr
                )

    def logits_half(h, x_dt_ap):
        for k in range(2):
            tcn = 2 * h + k
            lg = ps.tile([128, VOCAB], F32, tag="mm", bufs=2, name="lg")
            for dc in range(2):
                nc.tensor.matmul(
                    out=lg,
                    lhsT=x_dt_ap[:, dc * T + tcn * 128 : dc * T + (tcn + 1) * 128],
                    rhs=lmht[:, dc * VOCAB : (dc + 1) * VOCAB],
                    start=(dc == 0), stop=(dc == 1),
                )
            lg_sb = ysp.tile([128, VOCAB], F32, tag="lg", name="lg_sb")
            nc.vector.tensor_copy(out=lg_sb, in_=lg)
            dma(out=out_e[tcn * 128 : (tcn + 1) * 128, :], in_=lg_sb)

    # ---------------- prologue: layer 0's h0 wave ----------------
    # phase1-h1 emitted after the left wave so layer 0's AG-h0 trigger
    # isn't queued behind 64 h1 matmuls.
    x_td, x_dt = x_first, xd_first
    phase1_half(0, x_dt)
    rope_half(0)
    P_ps = alloc_P()
    P_left(P_ps)
    mask_copy_left(P_ps)
    b1o0, yk_ps0 = ykv_ag(0, x_td)
    phase1_half(1, x_dt)

    for _layer in range(N_LAYER):
        last = _layer == N_LAYER - 1
        state = {}

        def mid_hook():
            state["ykv_td0"] = ykv_sum_ln(0, b1o0, pace=yk_ps0[:, 0:256])

        # LN-ykv-h0 rides mid-rope only once the collective path is warmed
        # up (for layer 0 the AG lands late; a mid-rope wait would stall
        # the whole DVE queue).
        rope_half(1, mid_hook=mid_hook if _layer > 0 else None)
        if "ykv_td0" not in state:
            state["ykv_td0"] = ykv_sum_ln(0, b1o0, pace=yk_ps0[:, 0:256])
        if dbg and _layer == 0:
            dma(out=dbg["xs"][:], in_=xsb)
            dma(out=dbg["qr"][:], in_=qrb)
        P_right(P_ps)
        mask_copy_right(P_ps)
        b1o1, yk_ps1 = ykv_ag(1, x_td)
        ykv_dt0 = ykv_tr(0, state["ykv_td0"])
        if dbg:
            dbg["_dump_st2"] = _layer == 0

        # LN-ykv-h1 rides the phase5(0) xy stream (AG-h1 lands well
        # before the hook point); its PE transposes follow phase5(0) in
        # the PE queue so phase5(1) starts without a serial LN latency.
        def p50_hook():
            state["ykv_td1"] = ykv_sum_ln(1, b1o1, pace=yk_ps1[:, 0:256])

        b2o0, _ = phase5(0, ykv_dt0, mid_hook=p50_hook)
        ykv_dt1 = ykv_tr(1, state["ykv_td1"])

        x_td_new = xp.tile([128, 4 * D], F16, tag="x_td", name="x_td")
        x_dt_new = xp.tile([128, 2 * T], F16, tag="x_dt", name="x_dt")
        b2o1, y_ps1 = phase5(1, ykv_dt1)
        if dbg:
            dbg["_dump_st2"] = False
        if dbg and _layer == 0:
            dma(out=dbg["Pb"][:], in_=Pb)
            dma(out=dbg["yk0"][:], in_=state["ykv_td0"])
            dma(out=dbg["yk1"][:], in_=state["ykv_td1"])
            dma(out=dbg["ykdt0"][:], in_=ykv_dt0)
            dma(out=dbg["ykdt1"][:], in_=ykv_dt1)
            dma(out=dbg["xy"][:], in_=xsb)

        # ---- tail: x update + next layer's h0 wave rides the AR windows
        x_upd_ln(0, b2o0, x_td, x_td_new,
                 dbg_key="ysum0" if dbg and _layer == 0 else None,
                 pace=y_ps1[0])
        x_upd_tr(0, x_td_new, x_dt_new)
        if not last:
            phase1_half(0, x_dt_new)

            def rope_hook():
                x_upd_ln(1, b2o1, x_td, x_td_new,
                         dbg_key="ysum1" if dbg and _layer == 0 else None,
                         pace=y_ps1[1])

            # x-upd-h1's LN chain rides at the end of the rope-h0 stream:
            # by chunk 7 AR-h1 has landed in steady state. For the first
            # layer the ARs land late, so keep it after the AG trigger.
            rope_half(0, mid_hook=rope_hook if _layer > 0 else None,
                      hook_at=7)
            P_ps = alloc_P()
            P_left(P_ps)
            mask_copy_left(P_ps)
            b1o0, yk_ps0 = ykv_ag(0, x_td_new)
            if _layer == 0:
                rope_hook()
            x_upd_tr(1, x_td_new, x_dt_new)
            phase1_half(1, x_dt_new)
        else:
            logits_half(0, x_dt_new)
            x_upd_ln(1, b2o1, x_td, x_td_new)
            x_upd_tr(1, x_td_new, x_dt_new)
            logits_half(1, x_dt_new)
        if dbg and _layer == 0:
            dma(out=dbg["xtd1"][:], in_=x_td_new)
        x_td, x_dt = x_td_new, x_dt_new


# ---------------------------------------------------------------- entry point

def kernel(idx, embed, encoder, encoder_v, decoder, lm_head):
    if "nc" not in _STATE:
        _STATE["nc"] = _build_bass()
    nc = _STATE["nc"]
    in_maps = _build_in_maps(idx, embed, encoder, encoder_v, decoder, lm_head)
    import os

    trace = bool(int(os.environ.get("KERNEL_TRACE", "0")))
    res = run_bass_kernel_spmd(nc, in_maps, core_ids=CORES, trace=trace)
    _STATE["last_results"] = res
    return res.results[0]["logits"].reshape(B, T, VOCAB).astype(np.float32)
